# revision 1
# baseline (speedup 1.0000x reference)
"""GAT layer kernel for Trainium2, 8 NeuronCores (edge-parallel by target range).

Strategy
--------
Reference computes, per edge (s -> t):
    e = leaky_relu(score_tgt[t] + score_src[s]); w = exp(e)
    out[t] = sum_e w * h_proj[s] / (sum_e w + 1e-16)      (per head)

The softmax denominator is constant within a target segment, so no per-edge
alpha is needed: each core accumulates `sum_e w*h_proj[s]` and `sum_e w` for
the target nodes it owns, then divides once per node.

Sharding: core r owns target nodes [r*npc, (r+1)*npc).  Host groups each
core's edges into 98 windows of 128 target nodes; within a window edges are
grouped by source-node quarter so the bulk gather can use the fast int16
`dma_gather` path (quarter size 2*npc = 25088 < 32768).

Device program (identical on all 8 cores, SPMD):
  Phase A: augmented matmul builds, for this core's node shard,
     shard_hps [npc,256] bf16 : [h_proj bf16 (128) | s_src f32 bitcast (8) | pad]
     shard_stp [npc,128] bf16 : [s_tgt f32 bitcast (8) | pad]
   shard_hps is AllGather-replicated to T_hps [8*npc,256]; shard_stp stays
   local (target scores are only read for this core's own windows).
  Phase B per window: dma_gather h_proj+s_src rows at src (4 calls, one per
   source quarter), dma_gather s_tgt rows at tgt from the local shard (1
   call); w = exp(leaky_relu(ss+st)); per 128-edge tile build a one-hot
   indicator from local target ids (is_equal vs iota) and matmul-accumulate
   weighted messages + denominators into PSUM; divide once per node; store.
"""

import math
import numpy as np

import concourse.bass as bass
import concourse.tile as tile
from concourse import bacc, mybir
from concourse.bass_utils import run_bass_kernel_spmd
from concourse.masks import make_identity

F32 = mybir.dt.float32
BF16 = mybir.dt.bfloat16
I16 = mybir.dt.int16

N_CORES = 8
H = 4          # heads
FO = 32        # per-head out features
C = H * FO     # 128
FI = 128       # in features
RB = 256       # padded gather row elems (bf16) for hp table: 512B
RS = 128       # padded row elems for st table: 256B
NQ = 4         # source quarters


class Cfg:
    def __init__(self, n_nodes, npc, n_win, Twq, Ow, Woff):
        self.n_nodes = n_nodes
        self.npc = npc                   # padded nodes per core
        self.n_win = n_win               # windows per core
        self.Twq = Twq                   # [n_win, NQ] tiles per (window, quarter)
        self.Ow = Ow                     # [n_win, NQ] tile offset of quarter in window
        self.Tw = Twq.sum(axis=1)        # [n_win] tiles per window
        self.Woff = Woff                 # [n_win] global tile offset of window
        self.tot_tiles = int(self.Tw.sum())
        self.n_total = npc * N_CORES
        self.QR = 2 * npc                # quarter size in rows


def build_program(cfg: Cfg, repeat: int = 1, single_core: bool = False, skip=()):
    nc = bacc.Bacc("TRN2", target_bir_lowering=False,
                   dynamic_dma_scratch_size=65536)
    npc, n_win, n_total = cfg.npc, cfg.n_win, cfg.n_total
    TT = cfg.tot_tiles

    h_shard = nc.declare_dram_parameter("h_shard", [npc, FI], F32, isOutput=False)
    W_p = nc.declare_dram_parameter("W", [FI, C], F32, isOutput=False)
    brow_p = nc.declare_dram_parameter("b_row", [1, C], F32, isOutput=False)
    bcol_p = nc.declare_dram_parameter("b_col", [C, 1], F32, isOutput=False)
    A8_p = nc.declare_dram_parameter("A8", [C, 2 * H], F32, isOutput=False)
    iota_p = nc.declare_dram_parameter("iota", [128, 128], F32, isOutput=False)
    qidx_p = nc.declare_dram_parameter("qidx", [128, TT * 8], I16, isOutput=False)
    stidx_p = nc.declare_dram_parameter("stidx", [128, TT * 8], I16, isOutput=False)
    tgl_p = nc.declare_dram_parameter("tgl", [128, TT], F32, isOutput=False)
    out_p = nc.declare_dram_parameter("out", [npc, C], F32, isOutput=True)

    shard_hps = nc.dram_tensor("shard_hps", [npc, RB], BF16)
    shard_stp = nc.dram_tensor("shard_stp", [npc, RS], BF16)
    T_hps = nc.dram_tensor("T_hps", [n_total, RB], BF16, addr_space="Shared")

    groups = [list(range(N_CORES))]
    ts = bass.ts

    with tile.TileContext(nc) as tc:
        with tc.tile_pool(name="const", bufs=1) as const:
            iota_bf = const.tile([128, 128], BF16)
            nc.gpsimd.dma_start(out=iota_bf[:], in_=iota_p[:, :])
            ident = const.tile([128, 128], F32)
            make_identity(nc, ident[:])
            ident_bf = const.tile([128, 128], BF16)
            make_identity(nc, ident_bf[:])
            W_aug = const.tile([FI, C + 2 * H], BF16)
            Bb = const.tile([128, C + 2 * H], F32)

            setup = tc.alloc_tile_pool(name="setup", bufs=1)
            with tc.tile_pool(name="psetup", bufs=1, space="PSUM") as psetup:
                # zero the never-computed pad columns of the gather tables
                Zpad = setup.tile([128, RB - (C + 2 * H)], BF16)
                nc.vector.memset(Zpad[:], 0.0)
                nzt = npc // 128
                nc.gpsimd.dma_start(
                    out=shard_hps[:, C + 2 * H:RB].rearrange(
                        "(n p) c -> n p c", p=128),
                    in_=Zpad[:].unsqueeze(1).to_broadcast(
                        [128, nzt, RB - (C + 2 * H)]))
                nc.gpsimd.dma_start(
                    out=shard_stp[:, 2 * H:RS].rearrange(
                        "(n p) c -> n p c", p=128),
                    in_=Zpad[:, :RS - 2 * H].unsqueeze(1).to_broadcast(
                        [128, nzt, RS - 2 * H]))

                W_sb = setup.tile([FI, C], BF16)
                nc.gpsimd.dma_start(out=W_sb[:], in_=W_p[:, :])

                A8_bf = setup.tile([C, 2 * H], BF16)
                nc.gpsimd.dma_start(out=A8_bf[:], in_=A8_p[:, :])
                psWt = psetup.tile([C, FI], BF16)
                nc.tensor.transpose(psWt[:], W_sb[:], ident_bf[:])
                rhs129 = setup.tile([C, FI + 1], BF16)
                nc.vector.tensor_copy(rhs129[:, 0:FI], psWt[:])
                nc.gpsimd.dma_start(out=rhs129[:, FI:FI + 1], in_=bcol_p[:, :])

                ps8 = psetup.tile([2 * H, FI + 1], F32)
                nc.tensor.matmul(ps8[:], A8_bf[:], rhs129[:], start=True, stop=True)

                S8 = setup.tile([2 * H, FI], F32)
                nc.vector.tensor_copy(S8[:], ps8[:, 0:FI])
                psT = psetup.tile([FI, 2 * H], F32)
                nc.tensor.transpose(psT[:], S8[:], ident[:2 * H, :2 * H])
                nc.vector.tensor_copy(W_aug[:, C:C + 2 * H], psT[:])

                C8 = setup.tile([2 * H, 1], F32)
                nc.vector.tensor_copy(C8[:], ps8[:, FI:FI + 1])
                psC = psetup.tile([1, 2 * H], F32)
                nc.tensor.transpose(psC[:], C8[:], ident[:2 * H, :2 * H])

                nc.vector.tensor_copy(W_aug[:, 0:C], W_sb[:])

                brow136 = setup.tile([1, C + 2 * H], F32)
                nc.sync.dma_start(out=brow136[:, 0:C], in_=brow_p[:, :])
                nc.vector.tensor_copy(brow136[:, C:C + 2 * H], psC[:])
                ones = setup.tile([1, 128], F32)
                nc.vector.memset(ones[:], 1.0)
                psBB = psetup.tile([128, C + 2 * H], F32)
                nc.tensor.matmul(psBB[:], ones[:], brow136[:], start=True, stop=True)
                nc.vector.tensor_copy(Bb[:], psBB[:])

            # Phase A: tables for this core's shard
            pa = tc.alloc_tile_pool(name="pa", bufs=3)
            with tc.tile_pool(name="psa", bufs=2, space="PSUM") as psa, \
                 tc.tile_pool(name="psx", bufs=2, space="PSUM") as psx:
                for i in range(npc // 128):
                    X = pa.tile([128, 128], BF16)
                    nc.gpsimd.dma_start(out=X[:], in_=h_shard[ts(i, 128), :])
                    psX = psx.tile([128, 128], BF16)
                    nc.tensor.transpose(psX[:], X[:], ident_bf[:])
                    Xt = pa.tile([128, 128], BF16)
                    nc.vector.tensor_copy(Xt[:], psX[:])
                    psA = psa.tile([128, C + 2 * H], F32)
                    nc.tensor.matmul(psA[:], Xt[:], W_aug[:], start=True, stop=True)
                    hp = pa.tile([128, C], BF16)
                    nc.vector.tensor_add(hp[:], psA[:, 0:C], Bb[:, 0:C])
                    sc = pa.tile([128, 2 * H], F32)
                    nc.vector.tensor_add(sc[:], psA[:, C:], Bb[:, C:])
                    nc.sync.dma_start(out=shard_hps[ts(i, 128), 0:C], in_=hp[:])
                    nc.sync.dma_start(
                        out=shard_hps[ts(i, 128), C:C + 2 * H].bitcast(F32),
                        in_=sc[:, 0:H])
                    nc.sync.dma_start(
                        out=shard_stp[ts(i, 128), 0:2 * H].bitcast(F32),
                        in_=sc[:, H:2 * H])

            tc.strict_bb_all_engine_barrier()
            if single_core:
                nc.gpsimd.dma_start(out=T_hps[0:npc, :], in_=shard_hps[:, :])
            else:
                nc.gpsimd.collective_compute(
                    "AllGather", mybir.AluOpType.bypass, replica_groups=groups,
                    ins=[shard_hps[:, :]], outs=[T_hps[:, :]])
            tc.strict_bb_all_engine_barrier()

            # ---------------- Phase B ----------------
            pb = tc.alloc_tile_pool(name="pb", bufs=3)
            pg = tc.alloc_tile_pool(name="pg", bufs=3)
            pi = tc.alloc_tile_pool(name="pi", bufs=3)
            with tc.tile_pool(name="psm", bufs=2, space="PSUM") as psm, \
                 tc.tile_pool(name="psd", bufs=2, space="PSUM") as psd:
              for _rep in range(repeat):
                for w in range(n_win):
                    Tw = int(cfg.Tw[w])
                    Wo = int(cfg.Woff[w])
                    QI = pb.tile([128, Tw * 8], I16, tag="QI")
                    nc.sync.dma_start(out=QI[:], in_=qidx_p[:, Wo * 8:(Wo + Tw) * 8])
                    SI = pb.tile([128, Tw * 8], I16, tag="SI")
                    nc.sync.dma_start(out=SI[:], in_=stidx_p[:, Wo * 8:(Wo + Tw) * 8])
                    TGL = pb.tile([128, Tw], F32, tag="TGL")
                    nc.sync.dma_start(out=TGL[:], in_=tgl_p[:, Wo:Wo + Tw])

                    G = pg.tile([128, Tw * RB], BF16, tag="G")
                    for q in range(NQ if "hp_gather" not in skip else 0):
                        Tq = int(cfg.Twq[w, q])
                        if Tq == 0:
                            continue
                        Oq = int(cfg.Ow[w, q])
                        dst = G[:, Oq * RB:(Oq + Tq) * RB].rearrange(
                            "p (t c) -> p t c", c=RB)
                        nc.gpsimd.dma_gather(
                            dst, T_hps[q * cfg.QR:(q + 1) * cfg.QR, :],
                            QI[:, Oq * 8:(Oq + Tq) * 8],
                            Tq * 128, Tq * 128, RB, single_packet=False)
                    Gst = pg.tile([128, Tw * RS], BF16, tag="Gst")
                    if "st_gather" not in skip:
                        nc.gpsimd.dma_gather(
                            Gst[:].rearrange("p (t c) -> p t c", c=RS),
                            shard_stp[:, :], SI[:], Tw * 128, Tw * 128, RS,
                            single_packet=False)
                    elif w == 0 and _rep == 0:
                        nc.vector.memset(Gst[:], 0.0)

                    ssv = G[:].rearrange("p (t c) -> p t c", c=RB)[
                        :, :, C:C + 2 * H].bitcast(F32)
                    stv = Gst[:].rearrange("p (t c) -> p t c", c=RS)[
                        :, :, 0:2 * H].bitcast(F32)
                    WB = pb.tile([128, Tw * H], BF16, tag="WB")
                    if "smallops" not in skip:
                        E = pb.tile([128, Tw * H], F32, tag="E")
                        e3 = E[:].rearrange("p (t h) -> p t h", h=H)
                        nc.vector.tensor_tensor(out=e3, in0=ssv, in1=stv,
                                                op=mybir.AluOpType.add)
                        E2 = pb.tile([128, Tw * H], F32, tag="E2")
                        nc.vector.scalar_tensor_tensor(
                            E2[:], E[:], 0.2, E[:],
                            op0=mybir.AluOpType.mult, op1=mybir.AluOpType.max)
                        Wf = pb.tile([128, Tw * H], F32, tag="Wf")
                        nc.scalar.activation(Wf[:], E2[:],
                                             mybir.ActivationFunctionType.Exp)
                        nc.vector.tensor_copy(WB[:], Wf[:])
                    elif w == 0 and _rep == 0:
                        nc.vector.memset(WB[:], 1.0)

                    ps_m = psm.tile([128, C], F32)
                    ps_d = psd.tile([128, H], F32)
                    for t in range(Tw):
                        IND = pi.tile([128, 128], BF16, tag="IND")
                        MSG = pi.tile([128, C], BF16, tag="MSG")
                        if "dvetile" not in skip:
                            nc.vector.tensor_scalar(
                                IND[:], iota_bf[:], TGL[:, t:t + 1], None,
                                mybir.AluOpType.is_equal)
                            g3 = G[:, t * RB:t * RB + C].rearrange(
                                "p (h f) -> p h f", h=H)
                            wb3 = WB[:, t * H:(t + 1) * H].unsqueeze(2).to_broadcast(
                                [128, H, FO])
                            m3 = MSG[:].rearrange("p (h f) -> p h f", h=H)
                            nc.vector.tensor_tensor(
                                out=m3, in0=g3, in1=wb3, op=mybir.AluOpType.mult)
                        elif w == 0 and t < 3 and _rep == 0:
                            nc.vector.memset(IND[:], 0.0)
                            nc.vector.memset(MSG[:], 0.0)
                        if "mm" not in skip:
                            nc.tensor.matmul(ps_m[:], IND[:], MSG[:],
                                             start=(t == 0), stop=(t == Tw - 1))
                            nc.tensor.matmul(ps_d[:], IND[:],
                                             WB[:, t * H:(t + 1) * H],
                                             start=(t == 0), stop=(t == Tw - 1))

                    if "mm" in skip:
                        nc.vector.memset(ps_m[:], 0.0)
                        nc.vector.memset(ps_d[:], 0.0)
                    DEN = pb.tile([128, H], F32, tag="DEN")
                    nc.vector.tensor_scalar(
                        DEN[:], ps_d[:], 1e-16, None, mybir.AluOpType.add)
                    RCP = pb.tile([128, H], F32, tag="RCP")
                    nc.vector.reciprocal(RCP[:], DEN[:])
                    O = pb.tile([128, C], F32, tag="O")
                    o3 = O[:].rearrange("p (h f) -> p h f", h=H)
                    pm3 = ps_m[:].rearrange("p (h f) -> p h f", h=H)
                    r3 = RCP[:].unsqueeze(2).to_broadcast([128, H, FO])
                    nc.vector.tensor_tensor(
                        out=o3, in0=pm3, in1=r3, op=mybir.AluOpType.mult)
                    nc.sync.dma_start(out=out_p[ts(w, 128), :], in_=O[:])

            for _pool in (pi, pg, pb, pa, setup):
                _pool.release()

    if not nc.is_finalized():
        nc.finalize()
    return nc


# ---------------------------------------------------------------------------
# host side
# ---------------------------------------------------------------------------

def _wrap16(vals, n_slots):
    """Slot s -> [s % 16, s // 16], replicated across the 8 Q7 groups."""
    a = np.zeros((16, n_slots // 16), dtype=np.int16)
    a[np.arange(len(vals)) % 16, np.arange(len(vals)) // 16] = vals
    return np.tile(a, (8, 1))


def compute_cfg(edge_index, n_nodes):
    npc = int(math.ceil(n_nodes / (N_CORES * 128))) * 128
    n_win = npc // 128
    QR = 2 * npc
    src = np.asarray(edge_index[0], dtype=np.int64)
    tgt = np.asarray(edge_index[1], dtype=np.int64)
    core = tgt // npc
    loc = tgt - core * npc
    w = loc >> 7
    q = src // QR
    # counts[r, w, q]
    flat = (core * n_win + w) * NQ + q
    counts = np.bincount(flat, minlength=N_CORES * n_win * NQ).reshape(
        N_CORES, n_win, NQ)
    mx = counts.max(axis=0)                       # [n_win, NQ]
    Twq = np.ceil(mx / 128).astype(np.int64)      # tiles per (w, q)
    Ow = np.concatenate(
        [np.zeros((n_win, 1), np.int64), np.cumsum(Twq, axis=1)[:, :-1]], axis=1)
    Tw = Twq.sum(axis=1)
    Woff = np.concatenate([[0], np.cumsum(Tw)[:-1]])
    return Cfg(n_nodes, npc, n_win, Twq, Ow, Woff)


def prep_inputs(h_in, edge_index, W, b, a_src, a_tgt, cfg: Cfg):
    npc, n_win, QR = cfg.npc, cfg.n_win, cfg.QR
    TT = cfg.tot_tiles
    src = np.asarray(edge_index[0], dtype=np.int64)
    tgt = np.asarray(edge_index[1], dtype=np.int64)

    h_in = np.asarray(h_in, dtype=np.float32)
    W = np.asarray(W, dtype=np.float32)
    b = np.asarray(b, dtype=np.float32).reshape(-1)
    a_src = np.asarray(a_src, dtype=np.float32)
    a_tgt = np.asarray(a_tgt, dtype=np.float32)

    A8 = np.zeros((C, 2 * H), dtype=np.float32)
    for h in range(H):
        A8[h * FO:(h + 1) * FO, h] = a_src[h]
        A8[h * FO:(h + 1) * FO, H + h] = a_tgt[h]
    iota = np.tile(np.arange(128, dtype=np.float32), (128, 1))

    h_pad = np.zeros((cfg.n_total, FI), dtype=np.float32)
    h_pad[:cfg.n_nodes] = h_in

    # global tile slot base per (w, q)
    slot_base = (cfg.Woff[:, None] + cfg.Ow) * 128      # [n_win, NQ]

    core = tgt // npc
    in_maps = []
    for r in range(N_CORES):
        m = core == r
        s_r = src[m]
        loc = tgt[m] - r * npc
        w_r = loc >> 7
        q_r = s_r // QR
        # assign slots: order by (w, q), then sequential within group
        key = w_r * NQ + q_r
        order = np.argsort(key, kind="stable")
        s_r, loc, w_r, q_r, key = (a[order] for a in (s_r, loc, w_r, q_r, key))
        cnt = np.bincount(key, minlength=n_win * NQ)
        starts = np.concatenate([[0], np.cumsum(cnt)[:-1]])
        pos = np.arange(len(key)) - np.repeat(starts, cnt)
        slot = slot_base[w_r, q_r] + pos

        qidx_flat = np.zeros(TT * 128, dtype=np.int16)
        stidx_flat = np.zeros(TT * 128, dtype=np.int16)
        tgl_flat = np.full(TT * 128, -1.0, dtype=np.float32)
        # defaults for padded slots: qidx 0 (valid row of the quarter),
        # stidx = window's first node (valid), tgl -1 (no indicator match)
        for w in range(n_win):
            sl = slice(int(cfg.Woff[w]) * 128, int(cfg.Woff[w] + cfg.Tw[w]) * 128)
            stidx_flat[sl] = w * 128
        qidx_flat[slot] = (s_r - q_r * QR).astype(np.int16)
        stidx_flat[slot] = loc.astype(np.int16)
        tgl_flat[slot] = (loc & 127).astype(np.float32)

        # pack: per-window 16-wrap for idx arrays; [p, tile] for tgl
        qidx = np.zeros((128, TT * 8), dtype=np.int16)
        stidx = np.zeros((128, TT * 8), dtype=np.int16)
        tgl = np.zeros((128, TT), dtype=np.float32)
        for w in range(n_win):
            Wo, Tw = int(cfg.Woff[w]), int(cfg.Tw[w])
            # qidx wraps per-quarter-group (gather calls are per quarter)
            for q in range(NQ):
                Tq = int(cfg.Twq[w, q])
                if Tq == 0:
                    continue
                Oq = int(cfg.Ow[w, q])
                base = (Wo + Oq) * 128
                vals = qidx_flat[base:base + Tq * 128]
                qidx[:, (Wo + Oq) * 8:(Wo + Oq + Tq) * 8] = _wrap16(vals, Tq * 128)
            vals = stidx_flat[Wo * 128:(Wo + Tw) * 128]
            stidx[:, Wo * 8:(Wo + Tw) * 8] = _wrap16(vals, Tw * 128)
            tgl[:, Wo:Wo + Tw] = tgl_flat[Wo * 128:(Wo + Tw) * 128].reshape(
                Tw, 128).T

        in_maps.append({
            "h_shard": np.ascontiguousarray(h_pad[r * npc:(r + 1) * npc]),
            "W": W,
            "b_row": b.reshape(1, C),
            "b_col": b.reshape(C, 1),
            "A8": A8,
            "iota": iota,
            "qidx": qidx,
            "stidx": stidx,
            "tgl": tgl,
        })
    return in_maps


_prog_cache = {}


def kernel(h_in, edge_index, W, b, a_src, a_tgt):
    n_nodes = h_in.shape[0]
    cfg = compute_cfg(edge_index, n_nodes)
    key = (n_nodes, cfg.npc, cfg.tot_tiles, tuple(cfg.Tw.tolist()))
    if key not in _prog_cache:
        _prog_cache[key] = build_program(cfg)
    nc = _prog_cache[key]
    in_maps = prep_inputs(h_in, edge_index, W, b, a_src, a_tgt, cfg)
    res = run_bass_kernel_spmd(nc, in_maps, list(range(N_CORES)))
    out = np.concatenate([res.results[r]["out"] for r in range(N_CORES)], axis=0)
    return np.ascontiguousarray(out[:n_nodes])



# revision 8
# speedup vs baseline: 1.6264x; 1.6264x over previous
"""GAT layer kernel for Trainium2, 8 NeuronCores (edge-parallel by target range).

v3: superwindow-batched hp gathers; NO st gather — per-edge target scores
come from a transposed-indicator matmul on the PE.

Per edge (s -> t):  e = leaky_relu(score_tgt[t] + score_src[s]); w = exp(e)
    out[t] = (sum_e w * h_proj[s]) / (sum_e w + 1e-16)     (per head)

Sharding: core r owns target nodes [r*npc, (r+1)*npc).  Host groups each
core's edges into superwindows of SW=3 windows (128 target nodes each);
within a superwindow tiles are ordered (quarter, window) so each of the 4
source-quarter hp gathers covers a contiguous tile range.

The only random-HBM traffic is the hp gather (512B rows:
[h_proj bf16 128 | s_src f32 bitcast 8 | pad]); random gather bandwidth is
the kernel's bottleneck, so everything else is computed on-chip:
  IND  [e, n] one-hot of each edge's target  (batched DVE is_equal)
  INDT [n, e] its transpose                  (ones-matmul PSUM broadcast of
       the slot-major target ids + batched DVE is_equal vs the partition
       index -- DVE lanes cannot read across partitions, PE can)
  st_e = INDT_t^T @ st_window                (4-col matmul per tile)
  w    = exp(leaky_relu(ss + st))            (batched; exp on Act engine)
  G[:, :128] *= w (in-place), w -> G pad cols [136:140)
  One matmul per tile: IND_t^T @ G[:, :140] accumulates messages AND
  denominators into PSUM [128, 140]; divide once per node; store.
"""

import math
import numpy as np
import ml_dtypes

import concourse.bass as bass
import concourse.tile as tile
from concourse import bacc, mybir
from concourse.bass_utils import run_bass_kernel_spmd
from concourse.masks import make_identity

F32 = mybir.dt.float32
BF16 = mybir.dt.bfloat16
I16 = mybir.dt.int16

N_CORES = 8
H = 4          # heads
FO = 32        # per-head out features
C = H * FO     # 128
FI = 128       # in features
RB = 256       # row elems (bf16) of the hp gather table: 512B
NQ = 4         # source quarters
SW = 3         # windows per superwindow
WB0 = C + 8    # col where per-edge exp-weights go in G (after ss bitcast)
MC = WB0 + H   # matmul moving cols: [msg 128 | ss junk 8 | wb 4] = 140
TCH = 4        # tiles per INDT-broadcast chunk (psum: 4*128 f32 = 2KB)


class Cfg:
    def __init__(self, n_nodes, npc, counts):
        self.n_nodes = n_nodes
        self.npc = npc
        self.n_win = npc // 128
        self.QR = 2 * npc
        self.n_total = npc * N_CORES
        self.Twq = np.ceil(counts / 128).astype(np.int64)   # [n_win, NQ]
        self.n_sw = (self.n_win + SW - 1) // SW
        self.sw_wins = [range(s * SW, min((s + 1) * SW, self.n_win))
                        for s in range(self.n_sw)]
        # layout order: (sw, q, w). tile_off[w, q] = first global tile slot.
        self.tile_off = np.zeros((self.n_win, NQ), dtype=np.int64)
        self.sw_off = np.zeros(self.n_sw, dtype=np.int64)    # global tile offs
        self.sw_T = np.zeros(self.n_sw, dtype=np.int64)
        self.swq_off = np.zeros((self.n_sw, NQ), dtype=np.int64)  # sw-local
        self.swq_T = np.zeros((self.n_sw, NQ), dtype=np.int64)
        off = 0
        for s in range(self.n_sw):
            self.sw_off[s] = off
            for q in range(NQ):
                self.swq_off[s, q] = off - self.sw_off[s]
                for w in self.sw_wins[s]:
                    self.tile_off[w, q] = off
                    off += int(self.Twq[w, q])
                self.swq_T[s, q] = off - self.sw_off[s] - self.swq_off[s, q]
            self.sw_T[s] = off - self.sw_off[s]
        self.tot_tiles = off
        # per window: sw-local tile indices in (q) order, for the matmuls
        self.win_tiles = [
            [int(self.tile_off[w, q] - self.sw_off[w // SW]) + t
             for q in range(NQ) for t in range(int(self.Twq[w, q]))]
            for w in range(self.n_win)
        ]
        # per sw: window-local index of each sw-local tile
        self.tile_win = []
        for s in range(self.n_sw):
            tw = np.zeros(int(self.sw_T[s]), dtype=np.int64)
            for wl, w in enumerate(self.sw_wins[s]):
                for t in self.win_tiles[w]:
                    tw[t] = wl
            self.tile_win.append(tw)

    def key(self):
        return (self.n_nodes, self.npc, self.tot_tiles,
                self.Twq.tobytes())


def build_program(cfg: Cfg, repeat: int = 1, single_core: bool = False,
                  skip=(), no_phase_b: bool = False, no_phase_a: bool = False,
                  rb: int | None = None):
    RB = rb or globals()["RB"]
    nc = bacc.Bacc("TRN2", target_bir_lowering=False,
                   dynamic_dma_scratch_size=65536, num_swdge_queues=4)
    npc, n_win, n_total = cfg.npc, cfg.n_win, cfg.n_total
    TT = cfg.tot_tiles

    h_shard = nc.declare_dram_parameter("h_shard", [npc, FI], F32, isOutput=False)
    W_p = nc.declare_dram_parameter("W", [FI, C], F32, isOutput=False)
    brow_p = nc.declare_dram_parameter("b_row", [1, C], F32, isOutput=False)
    bcol_p = nc.declare_dram_parameter("b_col", [C, 1], F32, isOutput=False)
    A8_p = nc.declare_dram_parameter("A8", [C, 2 * H], F32, isOutput=False)
    qidx_p = nc.declare_dram_parameter("qidx", [128, TT * 8], I16, isOutput=False)
    tgl_p = nc.declare_dram_parameter("tgl", [128, TT], BF16, isOutput=False)
    tglT_p = nc.declare_dram_parameter("tglT", [1, TT * 128], BF16, isOutput=False)
    iotac_p = nc.declare_dram_parameter("iotac", [128, 1], F32, isOutput=False)
    iotar_p = nc.declare_dram_parameter("iotar", [128, 128], BF16, isOutput=False)
    out_p = nc.declare_dram_parameter("out", [npc, C], F32, isOutput=True)

    shard_hps = nc.dram_tensor("shard_hps", [npc, RB], BF16)
    st_tab = nc.dram_tensor("st_tab", [npc, H], BF16)
    T_hps = nc.dram_tensor("T_hps", [n_total, RB], BF16, addr_space="Shared")

    groups = [list(range(N_CORES))]
    ts = bass.ts

    with tile.TileContext(nc) as tc:
        with tc.tile_pool(name="const", bufs=1) as const:
            ident = const.tile([128, 128], F32)
            make_identity(nc, ident[:])
            ident_bf = const.tile([128, 128], BF16)
            make_identity(nc, ident_bf[:])
            W_aug = const.tile([FI, C + 2 * H], BF16)
            Bb = const.tile([128, C + 2 * H], F32)
            iota_bf = const.tile([128, 128], BF16)
            nc.gpsimd.dma_start(out=iota_bf[:], in_=iotar_p[:, :])
            iotac = const.tile([128, 1], F32)
            nc.gpsimd.dma_start(out=iotac[:], in_=iotac_p[:, :])
            ones_bf = const.tile([1, 128], BF16)
            nc.vector.memset(ones_bf[:], 1.0)

            setup = tc.alloc_tile_pool(name="setup", bufs=1)
            with tc.tile_pool(name="psetup", bufs=1, space="PSUM") as psetup:
                # zero the never-computed pad columns of the hp table
                if RB > C + 8:
                    Zpad = setup.tile([128, RB], BF16)
                    nc.vector.memset(Zpad[:], 0.0)
                    nzt = npc // 128
                    nc.gpsimd.dma_start(
                        out=shard_hps[:, C + 8:RB].rearrange(
                            "(n p) c -> n p c", p=128),
                        in_=Zpad[:, :RB - C - 8].unsqueeze(1).to_broadcast(
                            [128, nzt, RB - C - 8]))

                W_sb = setup.tile([FI, C], BF16)
                nc.gpsimd.dma_start(out=W_sb[:], in_=W_p[:, :])

                A8_bf = setup.tile([C, 2 * H], BF16)
                nc.gpsimd.dma_start(out=A8_bf[:], in_=A8_p[:, :])
                psWt = psetup.tile([C, FI], BF16)
                nc.tensor.transpose(psWt[:], W_sb[:], ident_bf[:])
                rhs129 = setup.tile([C, FI + 1], BF16)
                nc.vector.tensor_copy(rhs129[:, 0:FI], psWt[:])
                nc.gpsimd.dma_start(out=rhs129[:, FI:FI + 1], in_=bcol_p[:, :])

                ps8 = psetup.tile([2 * H, FI + 1], F32)
                nc.tensor.matmul(ps8[:], A8_bf[:], rhs129[:], start=True, stop=True)

                S8 = setup.tile([2 * H, FI], F32)
                nc.vector.tensor_copy(S8[:], ps8[:, 0:FI])
                psT = psetup.tile([FI, 2 * H], F32)
                nc.tensor.transpose(psT[:], S8[:], ident[:2 * H, :2 * H])
                nc.vector.tensor_copy(W_aug[:, C:C + 2 * H], psT[:])

                C8 = setup.tile([2 * H, 1], F32)
                nc.vector.tensor_copy(C8[:], ps8[:, FI:FI + 1])
                psC = psetup.tile([1, 2 * H], F32)
                nc.tensor.transpose(psC[:], C8[:], ident[:2 * H, :2 * H])

                nc.vector.tensor_copy(W_aug[:, 0:C], W_sb[:])

                brow136 = setup.tile([1, C + 2 * H], F32)
                nc.sync.dma_start(out=brow136[:, 0:C], in_=brow_p[:, :])
                nc.vector.tensor_copy(brow136[:, C:C + 2 * H], psC[:])
                ones = setup.tile([1, 128], F32)
                nc.vector.memset(ones[:], 1.0)
                psBB = psetup.tile([128, C + 2 * H], F32)
                nc.tensor.matmul(psBB[:], ones[:], brow136[:], start=True, stop=True)
                nc.vector.tensor_copy(Bb[:], psBB[:])

            # Phase A: hp table + local target-score table
            pa = tc.alloc_tile_pool(name="pa", bufs=3)
            with tc.tile_pool(name="psa", bufs=2, space="PSUM") as psa, \
                 tc.tile_pool(name="psx", bufs=2, space="PSUM") as psx:
                for i in range(npc // 128 if not no_phase_a else 0):
                    X = pa.tile([128, 128], BF16)
                    nc.gpsimd.dma_start(out=X[:], in_=h_shard[ts(i, 128), :])
                    psX = psx.tile([128, 128], BF16)
                    nc.tensor.transpose(psX[:], X[:], ident_bf[:])
                    Xt = pa.tile([128, 128], BF16)
                    nc.vector.tensor_copy(Xt[:], psX[:])
                    psA = psa.tile([128, C + 2 * H], F32)
                    nc.tensor.matmul(psA[:], Xt[:], W_aug[:], start=True, stop=True)
                    hp = pa.tile([128, C], BF16)
                    nc.vector.tensor_add(hp[:], psA[:, 0:C], Bb[:, 0:C])
                    sc = pa.tile([128, 2 * H], F32)
                    nc.vector.tensor_add(sc[:], psA[:, C:], Bb[:, C:])
                    nc.sync.dma_start(out=shard_hps[ts(i, 128), 0:C], in_=hp[:])
                    if RB >= C + 8:
                        nc.sync.dma_start(
                            out=shard_hps[ts(i, 128), C:C + 8].bitcast(F32),
                            in_=sc[:, 0:H])
                    stbf = pa.tile([128, H], BF16)
                    nc.vector.tensor_copy(stbf[:], sc[:, H:2 * H])
                    nc.sync.dma_start(out=st_tab[ts(i, 128), :], in_=stbf[:])

            if not no_phase_a:
                tc.strict_bb_all_engine_barrier()
                if single_core:
                    nc.gpsimd.dma_start(out=T_hps[0:npc, :], in_=shard_hps[:, :])
                else:
                    nc.gpsimd.collective_compute(
                        "AllGather", mybir.AluOpType.bypass,
                        replica_groups=groups,
                        ins=[shard_hps[:, :]], outs=[T_hps[:, :]])
                tc.strict_bb_all_engine_barrier()

            # ---------------- Phase B ----------------
            if no_phase_b:
                zo = const.tile([128, C], F32)
                nc.vector.memset(zo[:], 0.0)
                for w in range(n_win):
                    nc.sync.dma_start(out=out_p[ts(w, 128), :], in_=zo[:])
                for _pool in (pa, setup):
                    _pool.release()
            else:
              pb = tc.alloc_tile_pool(name="pb", bufs=3)
              pg = tc.alloc_tile_pool(name="pg", bufs=2)
              with tc.tile_pool(name="psm", bufs=3, space="PSUM") as psm, \
                   tc.tile_pool(name="psst", bufs=2, space="PSUM") as psst, \
                   tc.tile_pool(name="psbc", bufs=2, space="PSUM") as psbc:
                for _rep in range(repeat):
                  for s in range(cfg.n_sw):
                    T = int(cfg.sw_T[s])
                    Off = int(cfg.sw_off[s])
                    wins = cfg.sw_wins[s]
                    nw = len(wins)
                    QI = pb.tile([128, T * 8], I16, tag="QI")
                    nc.sync.dma_start(out=QI[:], in_=qidx_p[:, Off * 8:(Off + T) * 8])
                    TGL = pb.tile([128, T], BF16, tag="TGL")
                    nc.sync.dma_start(out=TGL[:], in_=tgl_p[:, Off:Off + T])
                    stw = pb.tile([128, nw * H], BF16, tag="stw")
                    nc.sync.dma_start(
                        out=stw[:].rearrange("p (w h) -> p w h", h=H),
                        in_=st_tab[wins[0] * 128:(wins[0] + nw) * 128, :]
                        .rearrange("(w p) h -> p w h", p=128))

                    G = pg.tile([128, T * RB], BF16, tag="G")
                    if "hp_gather" not in skip:
                        for q in range(NQ):
                            Tq = int(cfg.swq_T[s, q])
                            if Tq == 0:
                                continue
                            Oq = int(cfg.swq_off[s, q])
                            dst = G[:, Oq * RB:(Oq + Tq) * RB].rearrange(
                                "p (t c) -> p t c", c=RB)
                            nc.gpsimd.dma_gather(
                                dst, T_hps[q * cfg.QR:(q + 1) * cfg.QR, :],
                                QI[:, Oq * 8:(Oq + Tq) * 8],
                                Tq * 128, Tq * 128, RB, single_packet=False,
                                queue_num=q)
                    else:
                        nc.vector.memset(G[:], 0.0)

                    g3 = G[:].rearrange("p (t c) -> p t c", c=RB)
                    IND = pg.tile([128, T * 128], BF16, tag="IND")
                    INDT = pg.tile([128, T * 128], BF16, tag="INDT")
                    ps_st = psst.tile([128, T * H], F32)
                    if "smallops" not in skip:
                        # IND[e, t, n] = (tgl[e, t] == n)
                        nc.vector.tensor_tensor(
                            out=IND[:].rearrange("p (t n) -> p t n", n=128),
                            in0=iota_bf[:].unsqueeze(1).to_broadcast([128, T, 128]),
                            in1=TGL[:].unsqueeze(2).to_broadcast([128, T, 128]),
                            op=mybir.AluOpType.is_equal)
                        if "indt" not in skip:
                            # INDT[n, (t,e)] = (tglT[t,e] == n) via PE broadcast
                            for c0 in range(0, T, TCH):
                                cw = min(TCH, T - c0) * 128
                                TGLT = pb.tile([1, TCH * 128], BF16, tag="TGLT")
                                nc.scalar.dma_start(
                                    out=TGLT[:, 0:cw],
                                    in_=tglT_p[:, (Off + c0) * 128:
                                               (Off + c0) * 128 + cw])
                                ps_t = psbc.tile([128, TCH * 128], F32)
                                nc.tensor.matmul(
                                    ps_t[:, 0:cw], ones_bf[:],
                                    TGLT[:, 0:cw],
                                    start=True, stop=True)
                                nc.vector.tensor_tensor(
                                    out=INDT[:, c0 * 128:c0 * 128 + cw],
                                    in0=ps_t[:, 0:cw],
                                    in1=iotac[:].to_broadcast([128, cw]),
                                    op=mybir.AluOpType.is_equal)
                            # st_e = INDT_t^T @ stw_w(t)
                            if "stmm" not in skip:
                                for t in range(T):
                                    wl = int(cfg.tile_win[s][t])
                                    nc.tensor.matmul(
                                        ps_st[:, t * H:(t + 1) * H],
                                        INDT[:, t * 128:(t + 1) * 128],
                                        stw[:, wl * H:(wl + 1) * H],
                                        start=True, stop=True)
                            else:
                                nc.vector.memset(ps_st[:], 0.0)
                        else:
                            nc.vector.memset(ps_st[:], 0.0)

                        ssv = g3[:, :, C:C + 8].bitcast(F32)
                        E = pb.tile([128, T * H], F32, tag="E")
                        e3 = E[:].rearrange("p (t h) -> p t h", h=H)
                        nc.vector.tensor_tensor(
                            out=e3, in0=ssv,
                            in1=ps_st[:].rearrange("p (t h) -> p t h", h=H),
                            op=mybir.AluOpType.add)
                        E2 = pb.tile([128, T * H], F32, tag="E2")
                        nc.vector.scalar_tensor_tensor(
                            E2[:], E[:], 0.2, E[:],
                            op0=mybir.AluOpType.mult, op1=mybir.AluOpType.max)
                        Wf = pb.tile([128, T * H], F32, tag="Wf")
                        nc.scalar.activation(Wf[:], E2[:],
                                             mybir.ActivationFunctionType.Exp)
                        # exp-weights into G pad cols [WB0:WB0+H), bf16
                        nc.vector.tensor_copy(
                            g3[:, :, WB0:WB0 + H],
                            Wf[:].rearrange("p (t h) -> p t h", h=H))
                        # scale messages in place: G[:, :, 0:128] *= w
                        g4 = g3[:, :, 0:C].rearrange("p t (h f) -> p t h f", f=FO)
                        wb4 = g3[:, :, WB0:WB0 + H].unsqueeze(3).to_broadcast(
                            [128, T, H, FO])
                        nc.vector.tensor_tensor(out=g4, in0=g4, in1=wb4,
                                                op=mybir.AluOpType.mult)
                    else:
                        nc.vector.memset(IND[:], 0.0)

                    for w in wins:
                        tiles = cfg.win_tiles[w]
                        ps = psm.tile([128, MC], F32)
                        if "mm" not in skip and tiles:
                            for k, t in enumerate(tiles):
                                nc.tensor.matmul(
                                    ps[:], IND[:, t * 128:(t + 1) * 128],
                                    G[:, t * RB:t * RB + MC],
                                    start=(k == 0), stop=(k == len(tiles) - 1))
                        else:
                            nc.vector.memset(ps[:], 0.0)
                        DEN = pb.tile([128, H], F32, tag="DEN")
                        nc.vector.tensor_scalar(
                            DEN[:], ps[:, WB0:WB0 + H], 1e-16, None,
                            mybir.AluOpType.add)
                        RCP = pb.tile([128, H], F32, tag="RCP")
                        nc.vector.reciprocal(RCP[:], DEN[:])
                        O = pb.tile([128, C], F32, tag="O")
                        o3 = O[:].rearrange("p (h f) -> p h f", h=H)
                        pm3 = ps[:, 0:C].rearrange("p (h f) -> p h f", h=H)
                        r3 = RCP[:].unsqueeze(2).to_broadcast([128, H, FO])
                        nc.vector.tensor_tensor(
                            out=o3, in0=pm3, in1=r3, op=mybir.AluOpType.mult)
                        nc.sync.dma_start(out=out_p[ts(w, 128), :], in_=O[:])

              for _pool in (pg, pb, pa, setup):
                  _pool.release()

    if not nc.is_finalized():
        nc.finalize()
    return nc


# ---------------------------------------------------------------------------
# host side
# ---------------------------------------------------------------------------

def _wrap16(vals):
    """Slot s -> [s % 16, s // 16], replicated across the 8 Q7 groups."""
    n = len(vals)
    a = np.zeros((16, n // 16), dtype=np.int16)
    a[np.arange(n) % 16, np.arange(n) // 16] = vals
    return np.tile(a, (8, 1))


def compute_cfg(edge_index, n_nodes):
    npc = int(math.ceil(n_nodes / (N_CORES * 128))) * 128
    n_win = npc // 128
    QR = 2 * npc
    src = np.asarray(edge_index[0], dtype=np.int64)
    tgt = np.asarray(edge_index[1], dtype=np.int64)
    core = tgt // npc
    loc = tgt - core * npc
    w = loc >> 7
    q = src // QR
    flat = (core * n_win + w) * NQ + q
    counts = np.bincount(flat, minlength=N_CORES * n_win * NQ).reshape(
        N_CORES, n_win, NQ)
    return Cfg(n_nodes, npc, counts.max(axis=0))


def prep_inputs(h_in, edge_index, W, b, a_src, a_tgt, cfg: Cfg):
    npc, n_win, QR = cfg.npc, cfg.n_win, cfg.QR
    TT = cfg.tot_tiles
    src = np.asarray(edge_index[0], dtype=np.int64)
    tgt = np.asarray(edge_index[1], dtype=np.int64)

    h_in = np.asarray(h_in, dtype=np.float32)
    W = np.asarray(W, dtype=np.float32)
    b = np.asarray(b, dtype=np.float32).reshape(-1)
    a_src = np.asarray(a_src, dtype=np.float32)
    a_tgt = np.asarray(a_tgt, dtype=np.float32)

    A8 = np.zeros((C, 2 * H), dtype=np.float32)
    for h in range(H):
        A8[h * FO:(h + 1) * FO, h] = a_src[h]
        A8[h * FO:(h + 1) * FO, H + h] = a_tgt[h]

    h_pad = np.zeros((cfg.n_total, FI), dtype=np.float32)
    h_pad[:cfg.n_nodes] = h_in

    iotac = np.arange(128, dtype=np.float32).reshape(128, 1)
    iotar = np.tile(np.arange(128, dtype=np.float32),
                    (128, 1)).astype(ml_dtypes.bfloat16)

    core = tgt // npc
    in_maps = []
    for r in range(N_CORES):
        m = core == r
        s_r = src[m]
        loc = tgt[m] - r * npc
        w_r = loc >> 7
        q_r = s_r // QR
        # layout order (sw, q, w): sort edges by that key
        key = ((w_r // SW) * NQ + q_r) * SW + (w_r - (w_r // SW) * SW)
        order = np.argsort(key, kind="stable")
        s_r, loc, w_r, q_r, key = (a[order] for a in (s_r, loc, w_r, q_r, key))
        cnt = np.bincount(key, minlength=cfg.n_sw * NQ * SW)
        starts = np.concatenate([[0], np.cumsum(cnt)[:-1]])
        pos = np.arange(len(key)) - np.repeat(starts, cnt)
        slot = cfg.tile_off[w_r, q_r] * 128 + pos

        qidx_flat = np.zeros(TT * 128, dtype=np.int16)
        tgl_flat = np.full(TT * 128, -1.0, dtype=np.float32)
        qidx_flat[slot] = (s_r - q_r * QR).astype(np.int16)
        tgl_flat[slot] = (loc & 127).astype(np.float32)

        qidx = np.zeros((128, TT * 8), dtype=np.int16)
        for s in range(cfg.n_sw):
            So = int(cfg.sw_off[s])
            for q in range(NQ):
                Tq = int(cfg.swq_T[s, q])
                if Tq == 0:
                    continue
                Oq = So + int(cfg.swq_off[s, q])
                qidx[:, Oq * 8:(Oq + Tq) * 8] = _wrap16(
                    qidx_flat[Oq * 128:(Oq + Tq) * 128])

        tgl = np.ascontiguousarray(
            tgl_flat.reshape(TT, 128).T).astype(ml_dtypes.bfloat16)
        tglT = tgl_flat.reshape(1, TT * 128).astype(ml_dtypes.bfloat16)

        in_maps.append({
            "h_shard": np.ascontiguousarray(h_pad[r * npc:(r + 1) * npc]),
            "W": W,
            "b_row": b.reshape(1, C),
            "b_col": b.reshape(C, 1),
            "A8": A8,
            "qidx": qidx,
            "tgl": tgl,
            "tglT": tglT,
            "iotac": iotac,
            "iotar": iotar,
        })
    return in_maps


_prog_cache = {}


def kernel(h_in, edge_index, W, b, a_src, a_tgt):
    n_nodes = h_in.shape[0]
    cfg = compute_cfg(edge_index, n_nodes)
    key = cfg.key()
    if key not in _prog_cache:
        _prog_cache[key] = (cfg, build_program(cfg))
    cfg, nc = _prog_cache[key]
    in_maps = prep_inputs(h_in, edge_index, W, b, a_src, a_tgt, cfg)
    res = run_bass_kernel_spmd(nc, in_maps, list(range(N_CORES)))
    out = np.concatenate([res.results[r]["out"] for r in range(N_CORES)], axis=0)
    return np.ascontiguousarray(out[:n_nodes])
